# revision 33
# baseline (speedup 1.0000x reference)
"""Multi-head attention (B=4, T=2048, D=1024, H=16) on 8 TRN2 NeuronCores.

Sharding: batch x head-half (4 batches x 2 halves of 8 heads = 8 cores).
Each core projects Q/K/V for its 8 heads over the full 2048 tokens, runs
attention, and computes partial output projections against its half of Wo.
The tensor-parallel FC "all-reduce" is a host-side sum of the partials.

Per-core program (all matmul inputs bf16, fp32 PSUM accumulation):
  - K/Q projections produce head-dim-major K^T/Q^T [512 hd, 2048 tok].
  - V projection is per-head, producing token-major V with a ones column
    (vaug [tok, head, 65]) so P @ V_aug accumulates the softmax denominator
    in column 64 of the same PSUM tile.
  - Scores S = K_blk @ Q^T land as [128 ktok, 1024 q] PSUM tiles; one exp
    per tile (scalar engine) writes P directly as bf16.
  - PV is output-stationary: o[128 q, 65] accumulates over the 16 k-blocks
    with P as the stationary operand (F=65 per matmul at bf16 rate),
    halving PE cost versus the [65, q] orientation.
  - Normalization is a DVE reciprocal + per-partition scalar multiply, then
    a PE transpose (via identity) back to head-major for the FC.
  - The FC accumulates all 4 head-pair blocks in PSUM; it is split into two
    half-token chunks so the first (which depends only on the even-n PVs)
    overlaps the final exp window and only the second trails PV(15).
  - Emission interleaves projection quanta and PV into the gaps between
    score segments so the scalar engine's exp stream (the co-bottleneck at
    ~260us) starts early and runs with minimal gaps.

Host side: transposes inputs to feature-major bf16, slices weights per
head-half, runs SPMD on 8 cores, sums the two partial y per batch, and
adds the exact (bv @ Wo.T + bo) bias (attention rows sum to 1 so the value
bias passes through; bq/bk are zero in this problem).
"""
import numpy as np
from contextlib import ExitStack

import ml_dtypes

import concourse.bass as bass
import concourse.tile as tile
from concourse import bacc, mybir
from concourse.bass_utils import run_bass_kernel_spmd

F32 = mybir.dt.float32
BF16 = mybir.dt.bfloat16
NPBF16 = ml_dtypes.bfloat16

B = 4
T = 2048
D = 1024
H = 16
DK = 64
NCORES = 8
HLOC = 8           # heads per core
DHALF = 512        # hd dims per core
NKB = T // 128     # 16 key blocks
EXP_SCALE = 1.0 / np.sqrt(DK)


def _emit(nc):
    xq = nc.dram_tensor("xq", [D, T], BF16, kind="ExternalInput").ap()   # query^T
    xk = nc.dram_tensor("xk", [D, T], BF16, kind="ExternalInput").ap()   # key^T
    xv = nc.dram_tensor("xv", [D, T], BF16, kind="ExternalInput").ap()   # value^T
    wq = nc.dram_tensor("wq", [D, DHALF], BF16, kind="ExternalInput").ap()
    wk = nc.dram_tensor("wk", [D, DHALF], BF16, kind="ExternalInput").ap()
    wv = nc.dram_tensor("wv", [D, DHALF], BF16, kind="ExternalInput").ap()
    wo = nc.dram_tensor("wo", [DHALF, D], BF16, kind="ExternalInput").ap()
    ident = nc.dram_tensor("ident", [128, 128], BF16, kind="ExternalInput").ap()
    y = nc.dram_tensor("y", [T, D], F32, kind="ExternalOutput").ap()  # partial

    with tile.TileContext(nc) as tc, ExitStack() as ctx:
        res = ctx.enter_context(tc.tile_pool(name="res", bufs=1))
        otT = res.tile([128, 4, T], BF16)      # normalized attention out^T
        vaug = res.tile([128, NKB, HLOC, DK + 1], BF16)
        wot = res.tile([128, 4, D], BF16)      # Wo^T slice [(ki p) m -> p ki m]
        idt = res.tile([128, 128], BF16)
        nc.vector.memset(vaug[:, :, :, DK:DK + 1], 1.0)

        # K^T / Q^T blocks [128 hd, 2048 tok], two rotating slots per tag:
        # block b evicts block b-2, whose score readers are long emitted.
        ktq = ctx.enter_context(tc.tile_pool(name="ktq", bufs=2))
        kts, qts = {}, {}

        # Resident weight stages (K/Q needed across all 4 blocks).
        wst = ctx.enter_context(tc.tile_pool(name="wst", bufs=1))
        wk_s = wst.tile([128, 8, DHALF], BF16, name="wk_s", tag="wk")
        wq_s = wst.tile([128, 8, DHALF], BF16, name="wq_s", tag="wq")
        wv_s = wst.tile([128, 8, DHALF], BF16, name="wv_s", tag="wv")

        # Input staging: four quarter slots [128, 8, 512] carry xk -> xq ->
        # xv (xv pinned for the per-head V quanta); slot c is a 2-deep
        # eighth slab [128, 8, 256] feeding the block 2/3 re-load chains.
        xst = ctx.enter_context(tc.tile_pool(name="xst", bufs=1))

        # PSUM: "big" [128,1024]f32 x2 (scores + FC), "sm" [128,512]f32 x4
        # (projection tiles, PV accumulators, transposes).
        bigp = ctx.enter_context(tc.tile_pool(name="bigp", bufs=2, space="PSUM"))
        smp = ctx.enter_context(tc.tile_pool(name="smp", bufs=4, space="PSUM"))

        ptp = ctx.enter_context(tc.tile_pool(name="ptp", bufs=2))   # P bf16
        nrm = ctx.enter_context(tc.tile_pool(name="nrm", bufs=1))   # recip/obf
        evp = ctx.enter_context(tc.tile_pool(name="evp", bufs=1))   # fc evict

        def load_quarter(src, i, slot, eng):
            xs = xst.tile([128, 8, 512], BF16, name=f"x_{slot}", tag=slot)
            eng.dma_start(
                xs, src[:, i * 512:(i + 1) * 512]
                .rearrange("(ki p) t -> p ki t", p=128))
            return xs

        def kq_chain(w_s, blk, xs, width, dst, dstcol):
            """One 8-ki projection chain -> dst[:, dstcol:dstcol+width]."""
            ps = smp.tile([128, 512], F32, name="pps", tag="sm")
            for ki in range(8):
                nc.tensor.matmul(
                    ps[:, 0:width],
                    lhsT=w_s[:, ki, blk * 128:(blk + 1) * 128],
                    rhs=xs[:, ki, 0:width],
                    start=(ki == 0), stop=(ki == 7))
            nc.vector.tensor_copy(dst[:, dstcol:dstcol + width], ps[:, 0:width])

        # Block 2/3 re-load fills: DMA an eighth of xk/xq into the 2-deep c
        # slot at the start of a gap; the projection chain runs at gap end so
        # the transfer overlaps the gap's V/PV work.
        fill_q = []

        def fill_dma(blk, which, e):
            src = xk if which == "k" else xq
            xs = xst.tile([128, 8, 256], BF16, name="x_c", tag="c", bufs=2)
            nc.sync.dma_start(
                xs, src[:, e * 256:(e + 1) * 256]
                .rearrange("(ki p) t -> p ki t", p=128))
            fill_q.append((blk, which, e, xs))

        def fill_chain():
            blk, which, e, xs = fill_q.pop(0)
            w_s, tiles = (wk_s, kts) if which == "k" else (wq_s, qts)
            if blk not in tiles:
                tiles[blk] = ktq.tile([128, T], BF16,
                                      name=f"{which}t{blk}",
                                      tag="k" if which == "k" else "q")
            kq_chain(w_s, blk, xs, 256, tiles[blk], e * 256)

        def v_quantum(h, xv_slots):
            """V projection for one head: vaug[:, :, h, 0:64]."""
            for tb in range(NKB):
                xs = xv_slots[tb // 4]
                ps = smp.tile([128, 512], F32, name="vps", tag="sm")
                for ki in range(8):
                    nc.tensor.matmul(
                        ps[:, 0:DK],
                        lhsT=xs[:, ki, (tb % 4) * 128:(tb % 4 + 1) * 128],
                        rhs=wv_s[:, ki, h * DK:(h + 1) * DK],
                        start=(ki == 0), stop=(ki == 7))
                nc.vector.tensor_copy(vaug[:, tb, h, 0:DK], ps[:, 0:DK])

        pts = {}

        def scores_seg(n, kb_lo, kb_hi):
            """Score st + exp for (head, q-half) n over key blocks [lo, hi)."""
            h, qh = divmod(n, 2)
            blk, po = h // 2, (h % 2) * 64
            ktb, qtb = kts[blk], qts[blk]
            if n not in pts:
                pts[n] = ptp.tile([128, NKB, 1024], BF16,
                                  name=f"pt{n % 2}", tag="pt")
            pt = pts[n]
            for kb in range(kb_lo, kb_hi):
                st = bigp.tile([128, 1024], F32, name="st", tag="big")
                for c in range(2):
                    nc.tensor.matmul(
                        st[:, c * 512:(c + 1) * 512],
                        lhsT=ktb[po:po + 64, kb * 128:(kb + 1) * 128],
                        rhs=qtb[po:po + 64,
                                qh * 1024 + c * 512:qh * 1024 + (c + 1) * 512],
                        start=True, stop=True)
                nc.scalar.activation(
                    pt[:, kb, :], st[:],
                    mybir.ActivationFunctionType.Exp, scale=EXP_SCALE)

        def scores(n):
            scores_seg(n, 0, NKB)

        def pv(n):
            """PV + normalize + transpose for (head, q-half) n -> otT."""
            h, qh = divmod(n, 2)
            blk, po = h // 2, (h % 2) * 64
            pt = pts.pop(n)
            obfs = []
            for qb in range(8):
                o = smp.tile([128, 512], F32, name="ops", tag="sm")
                for kb in range(NKB):
                    nc.tensor.matmul(
                        o[:, 0:DK + 1],
                        lhsT=pt[:, kb, qb * 128:(qb + 1) * 128],
                        rhs=vaug[:, kb, h, :],
                        start=(kb == 0), stop=(kb == NKB - 1))
                rd = nrm.tile([128, 1], F32, name="rd", tag="rd", bufs=4)
                nc.vector.reciprocal(rd[:], o[:, DK:DK + 1])
                obf = nrm.tile([128, DK], BF16, name="obf", tag="obf", bufs=8)
                nc.vector.tensor_scalar_mul(obf[:], o[:, 0:DK], rd[:])
                obfs.append(obf)
            for qb in range(8):
                tpf = smp.tile([128, 512], F32, name="tps", tag="sm")
                tpb = tpf[0:DK, 0:DK].bitcast(BF16)  # [64, 128] bf16 view
                nc.tensor.transpose(tpb, obfs[qb][:], idt[:])
                nc.vector.tensor_copy(
                    otT[po:po + 64, blk,
                        qh * 1024 + qb * 128:qh * 1024 + (qb + 1) * 128], tpb)

        def fc_chunk(tbs):
            """Output projection for token blocks tbs (all 4 ki accumulated).
            The tb 0..7 chunk depends only on the even-n PVs, so it runs
            inside the last exp window; tb 8..15 trails PV(15). Evictions
            rotate through eight dead staging slots and stores fan out over
            three DMA queues so the tail is transfer-bandwidth bound."""
            ev_slots = ["s0", "s1", "s2", "s3", "wk", "wq", "c", "ev"]
            for tb in tbs:
                fp = bigp.tile([128, 1024], F32, name="fcp", tag="big")
                for ki in range(4):
                    for c in range(2):
                        nc.tensor.matmul(
                            fp[:, c * 512:(c + 1) * 512],
                            lhsT=otT[:, ki, tb * 128:(tb + 1) * 128],
                            rhs=wot[:, ki, c * 512:(c + 1) * 512],
                            start=(ki == 0), stop=(ki == 3))
                slot = ev_slots[tb % 8]
                pool = {"wk": wst, "wq": wst, "ev": evp}.get(slot, xst)
                ev = pool.tile([128, 1024], F32, name="ev", tag=slot,
                               bufs=2 if slot == "c" else 1)
                nc.vector.tensor_copy(ev[:], fp[:])
                if tb < 8:   # overlaps the final exp window: keep off scalar
                    eng = (nc.gpsimd, nc.sync)[tb % 2]
                else:        # post-exp: all three queues available
                    eng = (nc.gpsimd, nc.sync, nc.scalar)[tb % 3]
                eng.dma_start(y[tb * 128:(tb + 1) * 128, :], ev[:])

        # ---- emission schedule ----
        # Scores S(n)/PV(n) over n = 2*head + q-half; pt slot n%2 frees after
        # PV(n-2), which is always emitted just before S(n). Startup projects
        # K/Q blocks 0 AND 1 from the quarter slots (so no re-load deadline
        # crunch); fills for blocks 2/3 and the per-head V quanta pack the PE
        # gaps between score segments.
        nc.gpsimd.dma_start(wk_s, wk.rearrange("(ki p) m -> p ki m", p=128))
        nc.scalar.dma_start(idt, ident)
        nc.scalar.dma_start(wq_s, wq.rearrange("(ki p) m -> p ki m", p=128))
        kts[0] = ktq.tile([128, T], BF16, name="kt0", tag="k")
        kts[1] = ktq.tile([128, T], BF16, name="kt1", tag="k")
        qts[0] = ktq.tile([128, T], BF16, name="qt0", tag="q")
        qts[1] = ktq.tile([128, T], BF16, name="qt1", tag="q")
        # xk quarters stream on sync; xq quarters follow on scalar/gpsimd
        # (transfers overlap the K chains' WAR window). S0's score chunk for
        # key blocks [4i, 4i+4) only needs xk quarter i, so the first exp
        # fires as soon as two xk and two xq quarters have been projected.
        xkq = [load_quarter(xk, i, f"s{i}", nc.sync) for i in range(2)]
        for i in range(2):
            kq_chain(wk_s, 0, xkq[i], 512, kts[0], i * 512)
            kq_chain(wk_s, 1, xkq[i], 512, kts[1], i * 512)
        for i, eng in enumerate((nc.scalar, nc.scalar)):
            xs = load_quarter(xq, i, f"s{i}", eng)
            kq_chain(wq_s, 0, xs, 512, qts[0], i * 512)
            kq_chain(wq_s, 1, xs, 512, qts[1], i * 512)
        # Remaining projection chains ride inside S0's scalar-paced window
        # (S0 chunk [4i, 4i+4) only needs xk quarter i; the q2/q3 Q chains
        # only gate S1) so exp1 follows exp0 without a gap.
        scores_seg(0, 0, 4)
        xk_q2 = load_quarter(xk, 2, "s2", nc.sync)
        kq_chain(wk_s, 0, xk_q2, 512, kts[0], 1024)
        kq_chain(wk_s, 1, xk_q2, 512, kts[1], 1024)
        scores_seg(0, 4, 8)
        xq_q2 = load_quarter(xq, 2, "s2", nc.gpsimd)
        kq_chain(wq_s, 0, xq_q2, 512, qts[0], 1024)
        kq_chain(wq_s, 1, xq_q2, 512, qts[1], 1024)
        scores_seg(0, 8, 12)
        xk_q3 = load_quarter(xk, 3, "s3", nc.sync)
        kq_chain(wk_s, 0, xk_q3, 512, kts[0], 1536)
        kq_chain(wk_s, 1, xk_q3, 512, kts[1], 1536)
        scores_seg(0, 12, 16)
        xq_q3 = load_quarter(xq, 3, "s3", nc.scalar)
        kq_chain(wq_s, 0, xq_q3, 512, qts[0], 1536)
        kq_chain(wq_s, 1, xq_q3, 512, qts[1], 1536)
        nc.gpsimd.dma_start(wv_s, wv.rearrange("(ki p) m -> p ki m", p=128))
        nc.gpsimd.dma_start(wot, wo.rearrange("(ki p) m -> p ki m", p=128))
        xv_q = [load_quarter(xv, i, f"s{i}", nc.gpsimd) for i in range(4)]
        scores(1)

        # fills: block2 (g2-g5) then block3 (g6-g9), 4 eighths per gap with
        # DMA at gap start and chain at gap end. Fills cannot start before
        # g2: the ktq slot rotation overwrites block b-2, whose score
        # readers must already be emitted (S3 is emitted after g1).
        FILLS = ([(2, "k", e) for e in range(8)] +
                 [(2, "q", e) for e in range(8)] +
                 [(3, "k", e) for e in range(8)] +
                 [(3, "q", e) for e in range(8)])
        # number of fills resolved in each gap g0..g13 (sum = 32); block 2
        # must finish by g5 (S8), block 3 by g10 (S12) -- lighten g6-g9 to
        # reduce scalar-engine drift in the heavy middle gaps
        GAP_FILLS = [0, 0, 4, 4, 4, 4, 3, 3, 3, 3, 4, 0, 0, 0]
        fi = 0

        def gap(g):
            nonlocal fi
            nfill = GAP_FILLS[g]
            for blk, which, e in FILLS[fi:fi + nfill]:
                fill_dma(blk, which, e)
            fi += nfill
            if g < 8:
                v_quantum(g, xv_q)
            pv(g)
            for _ in range(nfill):
                fill_chain()

        for g in range(14):
            gap(g)
            scores(g + 2)
        pv(14)
        fc_chunk(range(0, 8))      # needs only even-n PVs: overlaps exp(15)
        pv(15)
        fc_chunk(range(8, 16))


_CACHED = None


def _build():
    global _CACHED
    if _CACHED is None:
        nc = bacc.Bacc("TRN2", target_bir_lowering=False, debug=False)
        _emit(nc)
        nc.compile()
        _CACHED = nc
    return _CACHED


def _run(inputs, trace=False, trace_kwargs=None):
    """Shard, run on 8 cores, gather. Returns (y, BassKernelResults)."""
    query, key, value = inputs["query"], inputs["key"], inputs["value"]
    Wq, Wk, Wv, Wo = inputs["Wq"], inputs["Wk"], inputs["Wv"], inputs["Wo"]
    bv, bo = inputs["bv"], inputs["bo"]

    f32 = np.float32
    wqT = np.asarray(Wq, f32).T.astype(NPBF16)   # [in, out]
    wkT = np.asarray(Wk, f32).T.astype(NPBF16)
    wvT = np.asarray(Wv, f32).T.astype(NPBF16)
    woT = np.asarray(Wo, f32).T.astype(NPBF16)   # [in(=hd), out]
    ident = np.eye(128, dtype=NPBF16)

    xqs = [np.asarray(query[b], f32).T.astype(NPBF16) for b in range(B)]
    xks = [np.asarray(key[b], f32).T.astype(NPBF16) for b in range(B)]
    xvs = [np.asarray(value[b], f32).T.astype(NPBF16) for b in range(B)]

    in_maps = []
    for c in range(NCORES):
        b, hh = divmod(c, 2)
        sl = slice(hh * DHALF, (hh + 1) * DHALF)
        in_maps.append({
            "xq": xqs[b], "xk": xks[b], "xv": xvs[b],
            "wq": np.ascontiguousarray(wqT[:, sl]),
            "wk": np.ascontiguousarray(wkT[:, sl]),
            "wv": np.ascontiguousarray(wvT[:, sl]),
            "wo": np.ascontiguousarray(woT[sl, :]),
            "ident": ident,
        })

    nc = _build()
    kw = {}
    if trace:
        kw["trace"] = True
        kw["trace_kwargs"] = trace_kwargs or {}
    res = run_bass_kernel_spmd(nc, in_maps, core_ids=list(range(NCORES)), **kw)

    # host-side tensor-parallel reduction + exact bias
    bias = (np.asarray(bv, f32) @ np.asarray(Wo, f32).T + np.asarray(bo, f32))
    yout = np.empty((B, T, D), dtype=f32)
    for b in range(B):
        yout[b] = res.results[2 * b]["y"] + res.results[2 * b + 1]["y"]
        yout[b] += bias[None, :]
    return yout, res


def kernel(**inputs):
    yv, _ = _run(inputs, trace=False)
    return yv
